# revision 1
# baseline (speedup 1.0000x reference)
"""MultiHeadAttention (qk-LayerNorm + RoPE) Trainium2 kernel, 8 NeuronCores.

Sharding: batch (4) x head-group (2x8 heads). Core c handles batch c//2,
heads 8*(c%2) .. 8*(c%2)+7. Each core computes QKV projections for its
batch restricted to its head group, per-head LayerNorm + rotary embedding,
attention, and a partial output projection over its 512 context channels.
A pairwise ReduceScatter sums the two partial o_proj results per batch and
leaves each core with half the rows; the host concatenates.

All matmuls run as float32r (tf32-like) on the PE. Attention is computed
with scores transposed ([s, t] layout) so softmax normalization can be
deferred: ctx_unnorm and sum-of-exp come from one matmul with a ones row
appended to V, and the per-token reciprocal is broadcast across partitions
with a K=1 matmul.
"""
import sys

for _p in ("/opt/trn_rl_repo", "/root/.axon_site", "/root/.axon_site/_ro/trn_rl_repo",
           "/root/.axon_site/_ro/pypackages"):
    if _p not in sys.path:
        sys.path.append(_p)

import numpy as np

import concourse.bass as bass
import concourse.tile as tile
from concourse import bacc, mybir
from concourse.bass_utils import run_bass_kernel_spmd
from concourse.masks import make_identity

F32 = mybir.dt.float32
F32R = mybir.dt.float32r
P = 128
B, L, C, H, D = 4, 1024, 1024, 16, 64
HC = 8          # heads per core
CG = HC * D     # 512 context channels per core
NT = L // P     # 8 token tiles
NCK = C // P    # 8 contraction tiles
THETA = 50000.0
EPS = 1e-5

_NC_CACHE = {}


def _build_nc():
    nc = bacc.Bacc("TRN2", target_bir_lowering=False, debug=False, num_devices=8)

    xT_d = nc.dram_tensor("xT", [C, L], F32, kind="ExternalInput")
    wqT_d = nc.dram_tensor("wqT", [C, CG], F32, kind="ExternalInput")
    wkT_d = nc.dram_tensor("wkT", [C, CG], F32, kind="ExternalInput")
    wvT_d = nc.dram_tensor("wvT", [C, CG], F32, kind="ExternalInput")
    woT_d = nc.dram_tensor("woT", [D, HC, C], F32, kind="ExternalInput")
    aq_d = nc.dram_tensor("aq", [L, D], F32, kind="ExternalInput")
    bq_d = nc.dram_tensor("bq", [L, D], F32, kind="ExternalInput")
    ak_d = nc.dram_tensor("ak", [L, D], F32, kind="ExternalInput")
    bk_d = nc.dram_tensor("bk", [L, D], F32, kind="ExternalInput")
    ones_d = nc.dram_tensor("ones64", [1, D], F32, kind="ExternalInput")
    out_d = nc.dram_tensor("out", [L // 2, C], F32, kind="ExternalOutput")

    with tile.TileContext(nc) as tc:
        with (
            tc.tile_pool(name="const", bufs=1) as constp,
            tc.tile_pool(name="w", bufs=1) as wpool,
            tc.tile_pool(name="big", bufs=1) as bigp,
            tc.tile_pool(name="xt", bufs=2) as xtp,
            tc.tile_pool(name="sq", bufs=1) as sqp,
            tc.tile_pool(name="scr", bufs=2) as scrp,
            tc.tile_pool(name="rope", bufs=2) as ropep,
            tc.tile_pool(name="stat", bufs=2) as statp,
            tc.tile_pool(name="exp", bufs=2) as expp,
            tc.tile_pool(name="fin", bufs=2) as finp,
            tc.tile_pool(name="dram", bufs=1, space="DRAM") as dram,
        ):
            ident = constp.tile([P, P], F32)
            make_identity(nc, ident)
            eps_t = constp.tile([P, 1], F32)
            nc.vector.memset(eps_t[:], EPS)
            ones_row = constp.tile([65, D], F32R)
            nc.sync.dma_start(ones_row[64:65, :], ones_d.ap().bitcast(F32R))

            aq_t = constp.tile([P, NT, D], F32)
            nc.sync.dma_start(aq_t[:], aq_d.ap().rearrange("(t p) d -> p t d", p=P))
            bq_t = constp.tile([P, NT, D], F32)
            nc.sync.dma_start(bq_t[:], bq_d.ap().rearrange("(t p) d -> p t d", p=P))
            ak_t = constp.tile([P, NT, D], F32)
            nc.sync.dma_start(ak_t[:], ak_d.ap().rearrange("(t p) d -> p t d", p=P))
            bk_t = constp.tile([P, NT, D], F32)
            nc.sync.dma_start(bk_t[:], bk_d.ap().rearrange("(t p) d -> p t d", p=P))

            # per-ck weight tiles so the first matmuls start after ~1.5MB of DMA
            wq_t, wk_t, wv_t = [], [], []
            for ck in range(NCK):
                for lst, nm, d_ in ((wq_t, "wq", wqT_d), (wk_t, "wk", wkT_d),
                                    (wv_t, "wv", wvT_d)):
                    t_ = wpool.tile([P, CG], F32R, tag=f"{nm}{ck}", name=f"{nm}{ck}")
                    nc.sync.dma_start(
                        t_[:],
                        d_.ap().rearrange("(k p) o -> p k o", p=P)[:, ck, :].bitcast(F32R))
                    lst.append(t_)

            # v with a ones column appended per head: [s_tile, j, head, 65]
            v_sb = bigp.tile([P, NT, HC, D + 1], F32R)
            nc.sync.dma_start(
                v_sb[:, :, :, D:D + 1].rearrange("p t h o -> p (t h) o"),
                ones_d.ap()[0:1, 0:1].rearrange("a b -> a b ()").to_broadcast(
                    (P, NT * HC, 1)).bitcast(F32R),
            )
            qT_pack = bigp.tile([P, HC // 2, L], F32R)
            kT_pack = bigp.tile([P, HC // 2, L], F32R)
            ctxT = bigp.tile([D, HC, L], F32R)

            # ---------------- Phase 1: QKV + LN + RoPE + transpose ----------
            with tc.tile_pool(name="ps1", bufs=2, space="PSUM") as ps1, \
                 tc.tile_pool(name="pst", bufs=2, space="PSUM") as pst:
                for ti in range(NT):
                    xt = xtp.tile([P, NCK, P], F32R)
                    nc.sync.dma_start(
                        xt[:],
                        xT_d.ap().rearrange("(k p) t -> p k t", p=P)[:, :, bass.ts(ti, P)].bitcast(F32R),
                    )
                    psq = ps1.tile([P, CG], F32, tag="psq")
                    psk = ps1.tile([P, CG], F32, tag="psk")
                    psv = ps1.tile([P, CG], F32, tag="psv")
                    for ps_, w_ in ((psq, wq_t), (psk, wk_t), (psv, wv_t)):
                        for ck in range(NCK):
                            nc.tensor.matmul(ps_[:], xt[:, ck, :], w_[ck][:],
                                             start=(ck == 0), stop=(ck == NCK - 1))

                    # v straight to SBUF (rounded to f32r); ACT engine to keep DVE free
                    nc.scalar.copy(
                        v_sb[:, ti, :, 0:D],
                        psv[:].rearrange("p (h d) -> p h d", d=D))

                    # LN stats for q and k: sums and sums of squares
                    stats = statp.tile([P, 4, HC], F32)
                    for i, ps_ in enumerate((psq, psk)):
                        nc.vector.reduce_sum(
                            stats[:, 2 * i, :], ps_[:].rearrange("p (h d) -> p h d", d=D),
                            axis=mybir.AxisListType.X)
                        sq = sqp.tile([P, CG], F32)
                        nc.scalar.square(sq[:], ps_[:])
                        nc.vector.reduce_sum(
                            stats[:, 2 * i + 1, :], sq[:].rearrange("p (h d) -> p h d", d=D),
                            axis=mybir.AxisListType.X)
                    mus = statp.tile([P, 2, HC], F32)
                    nc.vector.tensor_scalar_mul(mus[:], stats[:, 0::2, :], 1.0 / D)
                    ms2 = statp.tile([P, 2, HC], F32)
                    nc.vector.tensor_scalar_mul(ms2[:], stats[:, 1::2, :], 1.0 / D)
                    var = statp.tile([P, 2, HC], F32)
                    nc.vector.tensor_mul(var[:], mus[:], mus[:])
                    nc.vector.tensor_sub(var[:], ms2[:], var[:])
                    std = statp.tile([P, 2, HC], F32)
                    nc.scalar.activation(std[:], var[:], mybir.ActivationFunctionType.Sqrt,
                                         bias=eps_t[:])
                    invstd = statp.tile([P, 2, HC], F32)
                    nc.vector.reciprocal(invstd[:], std[:])
                    shift = statp.tile([P, 2, HC], F32)
                    nc.vector.tensor_mul(shift[:], mus[:], invstd[:])

                    for i, (ps_, a_t, b_t, dstpack) in enumerate(
                            ((psq, aq_t, bq_t, qT_pack), (psk, ak_t, bk_t, kT_pack))):
                        inv_b = invstd[:, i, :].rearrange("p h -> p h ()").to_broadcast((P, HC, D))
                        sh_b = shift[:, i, :].rearrange("p h -> p h ()").to_broadcast((P, HC, D))
                        a_b = a_t[:, ti, :].rearrange("p d -> p () d").to_broadcast((P, HC, D))
                        t1 = scrp.tile([P, HC, D], F32, tag="t1")
                        nc.vector.tensor_mul(t1[:], ps_[:].rearrange("p (h d) -> p h d", d=D), inv_b)
                        nc.vector.tensor_sub(t1[:], t1[:], sh_b)
                        rope = ropep.tile([P, HC, D], F32, tag=f"rope{i}")
                        nc.vector.tensor_mul(rope[:], t1[:], a_b)
                        r2 = scrp.tile([P, HC, D], F32, tag="r2")
                        h_ = D // 2
                        nc.vector.tensor_mul(
                            r2[:, :, 0:h_], t1[:, :, h_:D],
                            b_t[:, ti, 0:h_].rearrange("p d -> p () d").to_broadcast((P, HC, h_)))
                        nc.vector.tensor_mul(
                            r2[:, :, h_:D], t1[:, :, 0:h_],
                            b_t[:, ti, h_:D].rearrange("p d -> p () d").to_broadcast((P, HC, h_)))
                        nc.vector.tensor_add(rope[:], rope[:], r2[:])
                        for pr in range(HC // 2):
                            ps_t = pst.tile([P, P], F32)
                            nc.tensor.transpose(
                                ps_t[:],
                                rope[:, 2 * pr:2 * pr + 2, :].rearrange("p h d -> p (h d)"),
                                ident[:])
                            nc.scalar.copy(dstpack[:, pr, bass.ts(ti, P)], ps_t[:])

            # ---------------- Phase 2: attention per head -------------------
            with tc.tile_pool(name="pss", bufs=2, space="PSUM") as pssp, \
                 tc.tile_pool(name="psc", bufs=1, space="PSUM") as pscp, \
                 tc.tile_pool(name="psr", bufs=1, space="PSUM") as psrp:
                for h in range(HC):
                    pr, sub = h // 2, h % 2
                    lo, hi = D * sub, D * sub + D
                    psc = pscp.tile([D + 1, L], F32)
                    for j in range(NT):
                        pss = pssp.tile([P, L], F32)
                        for m in range(2):
                            nc.tensor.matmul(
                                pss[:, bass.ts(m, 512)],
                                kT_pack[lo:hi, pr, bass.ts(j, P)],
                                qT_pack[lo:hi, pr, bass.ts(m, 512)],
                                start=True, stop=True)
                        expT = expp.tile([P, L], F32R)
                        nc.scalar.activation(expT[:], pss[:],
                                             mybir.ActivationFunctionType.Exp,
                                             scale=float(D) ** -0.5)
                        for m in range(2):
                            nc.tensor.matmul(
                                psc[:, bass.ts(m, 512)],
                                v_sb[:, j, h, :],
                                expT[:, bass.ts(m, 512)],
                                start=(j == 0), stop=(j == NT - 1))
                    recip = finp.tile([D + 1, L], F32R, tag="recip")
                    with nc.allow_low_precision(reason="f32r rounding for rb matmul"):
                        nc.vector.reciprocal(recip[D:D + 1, :], psc[D:D + 1, :])
                    ps_rb = psrp.tile([D, L], F32)
                    for m in range(2):
                        nc.tensor.matmul(
                            ps_rb[:, bass.ts(m, 512)],
                            ones_row[64:65, :],
                            recip[D:D + 1, bass.ts(m, 512)],
                            start=True, stop=True)
                    for m in range(2):
                        rb_sb = finp.tile([D, 512], F32, tag="rb")
                        nc.vector.tensor_copy(rb_sb[:], ps_rb[:, bass.ts(m, 512)])
                        nc.vector.tensor_mul(ctxT[:, h, bass.ts(m, 512)],
                                             psc[0:D, bass.ts(m, 512)], rb_sb[:])

            # ---------------- Phase 3: output projection --------------------
            # wo reuses the per-ck wq slots (dead after phase 1)
            wo_l = []
            for h in range(HC):
                wo_h = wpool.tile([D, C], F32R, tag=f"wq{h}", name=f"wo{h}")
                nc.sync.dma_start(wo_h[:], woT_d.ap()[:, h, :].bitcast(F32R))
                wo_l.append(wo_h)

            bounce_in = [dram.tile([L // 2, C], F32, tag=f"bin{i}", name=f"bin{i}")
                         for i in range(2)]
            bounce_out = [dram.tile([L // 4, C], F32, tag=f"bout{i}", name=f"bout{i}")
                         for i in range(2)]

            def emit_rs(half):
                nc.gpsimd.collective_compute(
                    "ReduceScatter",
                    mybir.AluOpType.add,
                    replica_groups=[[0, 1], [2, 3], [4, 5], [6, 7]],
                    ins=[bounce_in[half][:].opt()],
                    outs=[bounce_out[half][:].opt()],
                )
                nc.sync.dma_start(out_d.ap()[bass.ts(half, L // 4), :],
                                  bounce_out[half][:])

            with tc.tile_pool(name="pso", bufs=2, space="PSUM") as psop:
                for ti in range(NT):
                    pso = psop.tile([P, C], F32)
                    for m in range(2):
                        for h in range(HC):
                            nc.tensor.matmul(
                                pso[:, bass.ts(m, 512)],
                                ctxT[:, h, bass.ts(ti, P)],
                                wo_l[h][:, bass.ts(m, 512)],
                                start=(h == 0), stop=(h == HC - 1))
                    out_sb = finp.tile([P, C], F32, tag="out", bufs=1)
                    nc.vector.tensor_copy(out_sb[:], pso[:])
                    nc.sync.dma_start(bounce_in[ti // 4][bass.ts(ti % 4, P), :], out_sb[:])
                    if ti == NT // 2 - 1:
                        emit_rs(0)
                emit_rs(1)

    nc.compile()
    return nc


def _rope_tables(w, b):
    """A[t,d], B[t,d] with the rotate-half sign folded into B."""
    inv_freq = 1.0 / THETA ** (np.arange(0, D, 2, dtype=np.float64) / D)
    freqs = np.arange(L, dtype=np.float64)[:, None] * inv_freq[None, :]
    freqs = np.concatenate([freqs, freqs], axis=1)           # [L, D]
    cos, sin = np.cos(freqs), np.sin(freqs)
    w = w.astype(np.float64)
    w_rot = np.concatenate([w[D // 2:], w[:D // 2]])
    sgn = np.concatenate([-np.ones(D // 2), np.ones(D // 2)])
    A = (cos * w[None, :]).astype(np.float32)
    Bt = (sin * w_rot[None, :] * sgn[None, :]).astype(np.float32)
    if np.any(b != 0):
        raise NotImplementedError("nonzero qk-norm bias not supported")
    return A, Bt


def kernel(**inputs):
    x = np.asarray(inputs["q"], dtype=np.float32)
    Wq = np.asarray(inputs["Wq"], dtype=np.float32)
    Wk = np.asarray(inputs["Wk"], dtype=np.float32)
    Wv = np.asarray(inputs["Wv"], dtype=np.float32)
    Wo = np.asarray(inputs["Wo"], dtype=np.float32)
    bo = np.asarray(inputs["bo"], dtype=np.float32)
    assert not np.any(bo != 0), "nonzero output bias not supported"

    Aq, Bq = _rope_tables(np.asarray(inputs["qn_w"], np.float32),
                          np.asarray(inputs["qn_b"], np.float32))
    Ak, Bk = _rope_tables(np.asarray(inputs["kn_w"], np.float32),
                          np.asarray(inputs["kn_b"], np.float32))
    ones64 = np.ones((1, D), dtype=np.float32)
    WoT = np.ascontiguousarray(Wo.T)                          # [C(c'), C(o)]

    if "nc" not in _NC_CACHE:
        _NC_CACHE["nc"] = _build_nc()
    nc = _NC_CACHE["nc"]

    in_maps = []
    for c in range(8):
        b_, g = c // 2, c % 2
        sl = slice(g * CG, (g + 1) * CG)
        in_maps.append({
            "xT": np.ascontiguousarray(x[b_].T),
            "wqT": np.ascontiguousarray(Wq[sl, :].T),
            "wkT": np.ascontiguousarray(Wk[sl, :].T),
            "wvT": np.ascontiguousarray(Wv[sl, :].T),
            "woT": np.ascontiguousarray(
                WoT[sl, :].reshape(HC, D, C).transpose(1, 0, 2)),
            "aq": Aq, "bq": Bq, "ak": Ak, "bk": Bk,
            "ones64": ones64,
        })

    res = run_bass_kernel_spmd(nc, in_maps, core_ids=list(range(8)))
    # two half-ReduceScatters: each core's "out" holds [rank's quarter of rows
    # 0:512 ; rank's quarter of rows 512:1024]
    Q = L // 4
    out = np.empty((B, L, C), dtype=np.float32)
    for b_ in range(B):
        ev, od = res.results[2 * b_]["out"], res.results[2 * b_ + 1]["out"]
        out[b_, 0 * Q:1 * Q] = ev[0:Q]
        out[b_, 1 * Q:2 * Q] = od[0:Q]
        out[b_, 2 * Q:3 * Q] = ev[Q:2 * Q]
        out[b_, 3 * Q:4 * Q] = od[Q:2 * Q]
    return out



# revision 14
# speedup vs baseline: 1.7884x; 1.7884x over previous
"""MultiHeadAttention (qk-LayerNorm + RoPE) Trainium2 kernel, 8 NeuronCores.

Sharding: batch (4) x query-token-half (2x512 rows).  Core c handles batch
c//2 and output rows [512*(c%2), 512*(c%2)+512).  Each core computes K/V
projections for ALL 1024 tokens of its batch (duplicated within the pair --
cheaper than any collective: the baseline's ReduceScatter + device barrier
cost ~170us) and the Q projection for its 512 rows, so every core emits 512
complete output rows with ZERO cross-device communication.

Tricks:
- LN mean-centering is folded into Wq/Wk on the host (subtract per-head
  column means), so projections come out exactly mean-centered and only the
  variance is computed on-chip (ACT square + DVE reduce).
- rope(LN(x)) = inv * (x*A + rot(x)*B) since inv/shift are constant along d
  and shift == 0 -- one fused normalize+rope pass on DVE.
- D^-0.5 is folded into the q-side inv via bias = 64*eps, scale = 1 in the
  sqrt (giving 8*std directly).
- Attention per head-pair: scores for both heads land in one [128,1024]
  psum tile -> single 1024-wide exp.  V carries an appended ones column so
  ctx_unnorm and sum-of-exp come from one accumulated matmul; the odd head
  uses a 128-wide zero-padded V block so its ctx lands on psum partitions
  64:128 (sumexp on row 63), which keeps the o_proj contraction head-paired
  at K=128.
- All matmuls bf16 (PE upconverts to FP22 internally); psum f32.
"""
import sys

for _p in ("/opt/trn_rl_repo", "/root/.axon_site", "/root/.axon_site/_ro/trn_rl_repo",
           "/root/.axon_site/_ro/pypackages"):
    if _p not in sys.path:
        sys.path.append(_p)

import numpy as np
import ml_dtypes

import concourse.bass as bass
import concourse.tile as tile
from concourse import bacc, mybir
from concourse.bass_utils import run_bass_kernel_spmd
from concourse.masks import make_identity

F32 = mybir.dt.float32
F32R = mybir.dt.float32r
BF16 = mybir.dt.bfloat16
BF = ml_dtypes.bfloat16
P = 128
B, L, C, H, D = 4, 1024, 1024, 16, 64
NT = L // P        # 8 k/v token tiles
NQ = 4             # q token tiles (512 rows per core)
NCK = C // P       # 8 contraction chunks
NHP = H // 2       # 8 head pairs
QL = 512           # query rows per core
VW = 193           # v slot width per head pair: 65 (even) + 128 (odd)
THETA = 50000.0
EPS = 1e-5

_NC_CACHE = {}


def _build_nc():
    nc = bacc.Bacc("TRN2", target_bir_lowering=False, debug=False, num_devices=8)

    xT_d = nc.dram_tensor("xT", [C, L], BF16, kind="ExternalInput")
    wq_d = nc.dram_tensor("wq", [C, C], BF16, kind="ExternalInput")
    wk_d = nc.dram_tensor("wk", [C, C], BF16, kind="ExternalInput")
    wv_d = nc.dram_tensor("wv", [C, C], BF16, kind="ExternalInput")
    wo_d = nc.dram_tensor("wo", [NHP, P, C], BF16, kind="ExternalInput")
    aq_d = nc.dram_tensor("aq", [QL, D], F32, kind="ExternalInput")
    bq_d = nc.dram_tensor("bq", [QL, D], F32, kind="ExternalInput")
    ak_d = nc.dram_tensor("ak", [L, D], F32, kind="ExternalInput")
    bk_d = nc.dram_tensor("bk", [L, D], F32, kind="ExternalInput")
    onespat_d = nc.dram_tensor("onespat", [P, P], F32, kind="ExternalInput")
    out_d = nc.dram_tensor("out", [QL, C], F32, kind="ExternalOutput")

    with tile.TileContext(nc) as tc:
        with (
            tc.tile_pool(name="const", bufs=1) as constp,
            tc.tile_pool(name="w", bufs=1) as wpool,
            tc.tile_pool(name="big", bufs=1) as bigp,
            tc.tile_pool(name="sq", bufs=2) as sqp,
            tc.tile_pool(name="nrm", bufs=2) as nrmp,
            tc.tile_pool(name="st", bufs=2) as stp,
            tc.tile_pool(name="exp", bufs=3) as expp,
            tc.tile_pool(name="rb", bufs=2) as rbp,
            tc.tile_pool(name="rc", bufs=1) as rcp,
            tc.tile_pool(name="fin", bufs=2) as finp,
        ):
            ident = constp.tile([P, P], BF16)
            make_identity(nc, ident)
            eps_k = constp.tile([P, 1], F32)
            nc.vector.memset(eps_k[:], EPS)
            eps_q = constp.tile([P, 1], F32)
            nc.vector.memset(eps_q[:], float(D) * EPS)
            # ones_pat row 64 cols 0:64 = 1 (even-head recip broadcast),
            # row 32 cols 64:128 = 1 (odd-head recip broadcast to rows 64:128;
            # base partition must be 0/32/64 for the PE)
            ones_pat = constp.tile([P, P], F32R)
            nc.sync.dma_start(ones_pat[:], onespat_d.ap().bitcast(F32R))

            ak_t = constp.tile([P, NT, D], F32)
            nc.sync.dma_start(ak_t[:], ak_d.ap().rearrange("(t p) d -> p t d", p=P))
            bk_t = constp.tile([P, NT, D], F32)
            nc.sync.dma_start(bk_t[:], bk_d.ap().rearrange("(t p) d -> p t d", p=P))
            aq_t = constp.tile([P, NQ, D], F32)
            nc.sync.dma_start(aq_t[:], aq_d.ap().rearrange("(t p) d -> p t d", p=P))
            bq_t = constp.tile([P, NQ, D], F32)
            nc.sync.dma_start(bq_t[:], bq_d.ap().rearrange("(t p) d -> p t d", p=P))

            # per-ck chunks so matmuls start as soon as the first chunks land
            wk_l, wv_l, wq_l, xt_l = [], [], [], []
            for ck in range(NCK):
                for lst, nm, d_ in ((wk_l, "wk", wk_d), (wv_l, "wv", wv_d),
                                    (xt_l, "xt", xT_d)):
                    t_ = wpool.tile([P, C if nm != "xt" else L], BF16,
                                    tag=f"{nm}{ck}", name=f"{nm}{ck}")
                    nc.sync.dma_start(
                        t_[:], d_.ap().rearrange("(k p) o -> p k o", p=P)[:, ck, :])
                    lst.append(t_)
            for ck in range(NCK):
                t_ = wpool.tile([P, C], BF16, tag=f"wq{ck}", name=f"wq{ck}")
                nc.sync.dma_start(
                    t_[:], wq_d.ap().rearrange("(k p) o -> p k o", p=P)[:, ck, :])
                wq_l.append(t_)
            wo_t = wpool.tile([P, NHP, C], BF16, name="wo")
            nc.sync.dma_start(wo_t[:], wo_d.ap().rearrange("g p o -> p g o"))

            # v with per-pair slots [65 even | 128 odd]; odd block: col 32
            # ones, cols 64:128 = v, rest zero -> ctx lands on psum rows
            # 64:128 with sumexp on row 32.
            v_sb = bigp.tile([P, NT, NHP, VW], BF16)
            nc.vector.memset(v_sb[:, :, :, 65:129], 0.0)
            nc.vector.memset(v_sb[:, :, :, 64:65], 1.0)
            nc.vector.memset(v_sb[:, :, :, 97:98], 1.0)

            kT = bigp.tile([P, NHP, L], BF16)
            qT = bigp.tile([P, NHP, QL], BF16)
            ctxT = bigp.tile([P, NHP, QL], BF16)

            # ---------------- Phase 1: QKV + LN + RoPE + transpose ----------
            units = [("k", i) for i in range(NT)] + [("q", i) for i in range(NQ)]
            with tc.tile_pool(name="ps1", bufs=2, space="PSUM") as ps1, \
                 tc.tile_pool(name="pst", bufs=2, space="PSUM") as pst:
                for kind, ti in units:
                    is_k = kind == "k"
                    tok = ti * P
                    w_l = wk_l if is_k else wq_l
                    a_t, b_t = (ak_t, bk_t) if is_k else (aq_t, bq_t)
                    ps = ps1.tile([P, C], F32, tag="ps")
                    for m in range(2):
                        for ck in range(NCK):
                            nc.tensor.matmul(ps[:, bass.ts(m, 512)],
                                             xt_l[ck][:, tok:tok + P],
                                             w_l[ck][:, bass.ts(m, 512)],
                                             start=(ck == 0), stop=(ck == NCK - 1))
                    if is_k:
                        psv = ps1.tile([P, C], F32, tag="psv", bufs=1)
                        for m in range(2):
                            for ck in range(NCK):
                                nc.tensor.matmul(psv[:, bass.ts(m, 512)],
                                                 xt_l[ck][:, tok:tok + P],
                                                 wv_l[ck][:, bass.ts(m, 512)],
                                                 start=(ck == 0), stop=(ck == NCK - 1))
                    psr = ps[:].rearrange("p (h d) -> p h d", d=D)

                    # variance (mean is exactly 0: weights are pre-centered)
                    sq = sqp.tile([P, C], BF16, tag="sq")
                    nc.scalar.square(sq[:], ps[:])
                    vs = stp.tile([P, H], BF16, tag="vs")
                    with nc.allow_low_precision(reason="bf16 var accum, 0.4% rel"):
                        nc.vector.reduce_sum(
                            vs[:], sq[:].rearrange("p (h d) -> p h d", d=D),
                            axis=mybir.AxisListType.X)
                    std = stp.tile([P, H], F32, tag="std")
                    # k: sqrt(sum/64 + eps) = std ; q: sqrt(sum + 64eps) = 8*std
                    nc.scalar.activation(std[:], vs[:],
                                         mybir.ActivationFunctionType.Sqrt,
                                         bias=(eps_k[:] if is_k else eps_q[:]),
                                         scale=(1.0 / D if is_k else 1.0))
                    inv = stp.tile([P, H], F32, tag="inv")
                    nc.vector.reciprocal(inv[:], std[:])

                    # rope on raw centered ps, then scale by inv
                    nrm = nrmp.tile([P, H, D], BF16, tag="nrm")
                    tmp = nrmp.tile([P, H, D], BF16, tag="tmp", bufs=1)
                    a_b = a_t[:, ti, :].rearrange("p d -> p () d").to_broadcast((P, H, D))
                    nc.vector.tensor_mul(nrm[:], psr, a_b)
                    h_ = D // 2
                    nc.vector.tensor_mul(
                        tmp[:, :, 0:h_], psr[:, :, h_:D],
                        b_t[:, ti, 0:h_].rearrange("p d -> p () d").to_broadcast((P, H, h_)))
                    nc.vector.tensor_mul(
                        tmp[:, :, h_:D], psr[:, :, 0:h_],
                        b_t[:, ti, h_:D].rearrange("p d -> p () d").to_broadcast((P, H, h_)))
                    nc.vector.tensor_add(nrm[:], nrm[:], tmp[:])
                    inv_b = inv[:].rearrange("p h -> p h ()").to_broadcast((P, H, D))
                    nc.vector.tensor_mul(nrm[:], nrm[:], inv_b)

                    # transpose to [pair-channel, token]
                    dst = kT if is_k else qT
                    for g in range(2):
                        pt = pst.tile([P, 512], BF16, tag="pt")
                        for q4 in range(4):
                            hp = 4 * g + q4
                            nc.tensor.transpose(
                                pt[:, q4 * P:(q4 + 1) * P],
                                nrm[:, 2 * hp:2 * hp + 2, :].rearrange("p h d -> p (h d)"),
                                ident[:])
                        nc.scalar.copy(
                            dst[:, 4 * g:4 * g + 4, tok:tok + P],
                            pt[:].rearrange("p (f t) -> p f t", t=P))
                    if is_k:
                        psvr = psv[:].rearrange("p (g two d) -> p g two d", two=2, d=D)
                        nc.scalar.copy(v_sb[:, ti, :, 0:D], psvr[:, :, 0, :])
                        nc.scalar.copy(v_sb[:, ti, :, 129:VW], psvr[:, :, 1, :])

            # ---------------- Phase 2: attention per head pair --------------
            with tc.tile_pool(name="pss", bufs=2, space="PSUM") as pssp, \
                 tc.tile_pool(name="psc", bufs=2, space="PSUM") as pscp:
                for hp in range(NHP):
                    psc = pscp.tile([P, 2 * QL], F32, tag="psc")
                    for j in range(NT):
                        pss = pssp.tile([P, 2 * QL], F32, tag="pss")
                        jt = j * P
                        nc.tensor.matmul(pss[:, 0:QL],
                                         kT[0:64, hp, jt:jt + P], qT[0:64, hp, :],
                                         start=True, stop=True)
                        nc.tensor.matmul(pss[:, QL:2 * QL],
                                         kT[64:128, hp, jt:jt + P], qT[64:128, hp, :],
                                         start=True, stop=True)
                        ex = expp.tile([P, 2 * QL], BF16, tag="ex")
                        nc.scalar.activation(ex[:], pss[:],
                                             mybir.ActivationFunctionType.Exp)
                        nc.tensor.matmul(psc[0:65, 0:QL], v_sb[:, j, hp, 0:65],
                                         ex[:, 0:QL],
                                         start=(j == 0), stop=(j == NT - 1))
                        nc.tensor.matmul(psc[:, QL:2 * QL], v_sb[:, j, hp, 65:VW],
                                         ex[:, QL:2 * QL],
                                         start=(j == 0), stop=(j == NT - 1))
                    rc = rcp.tile([P, 2 * QL], F32R, tag="rc")
                    with nc.allow_low_precision(reason="f32r recip for rb matmul"):
                        nc.vector.reciprocal(rc[64:65, 0:QL], psc[64:65, 0:QL])
                        nc.vector.reciprocal(rc[32:33, QL:2 * QL],
                                             psc[32:33, QL:2 * QL])
                    rbps = pssp.tile([P, 2 * QL], F32, tag="pss")
                    nc.tensor.matmul(rbps[0:64, 0:QL], ones_pat[64:65, 0:64],
                                     rc[64:65, 0:QL], start=True, stop=True)
                    nc.tensor.matmul(rbps[:, QL:2 * QL], ones_pat[32:33, 0:P],
                                     rc[32:33, QL:2 * QL], start=True, stop=True)
                    rb = rbp.tile([P, 2 * QL], F32, tag="rb")
                    nc.vector.tensor_copy(rb[0:64, 0:QL], rbps[0:64, 0:QL])
                    nc.vector.tensor_copy(rb[64:128, QL:2 * QL],
                                          rbps[64:128, QL:2 * QL])
                    nc.vector.tensor_mul(ctxT[0:64, hp, :], psc[0:64, 0:QL],
                                         rb[0:64, 0:QL])
                    nc.vector.tensor_mul(ctxT[64:128, hp, :],
                                         psc[64:128, QL:2 * QL],
                                         rb[64:128, QL:2 * QL])

            # ---------------- Phase 3: output projection --------------------
            with tc.tile_pool(name="pso", bufs=2, space="PSUM") as psop:
                for qt in range(NQ):
                    pso = psop.tile([P, C], F32, tag="pso")
                    for m in range(2):
                        for g in range(NHP):
                            nc.tensor.matmul(pso[:, bass.ts(m, 512)],
                                             ctxT[:, g, qt * P:(qt + 1) * P],
                                             wo_t[:, g, bass.ts(m, 512)],
                                             start=(g == 0), stop=(g == NHP - 1))
                    osb = finp.tile([P, C], F32, tag="osb")
                    nc.vector.tensor_copy(osb[:], pso[:])
                    nc.sync.dma_start(out_d.ap()[qt * P:(qt + 1) * P, :], osb[:])

    nc.compile()
    return nc


def _rope_tables(w, b, n_tok):
    """A[t,d], B[t,d] with the rotate-half sign folded into B."""
    inv_freq = 1.0 / THETA ** (np.arange(0, D, 2, dtype=np.float64) / D)
    freqs = np.arange(n_tok, dtype=np.float64)[:, None] * inv_freq[None, :]
    freqs = np.concatenate([freqs, freqs], axis=1)           # [n_tok, D]
    cos, sin = np.cos(freqs), np.sin(freqs)
    w = w.astype(np.float64)
    w_rot = np.concatenate([w[D // 2:], w[:D // 2]])
    sgn = np.concatenate([-np.ones(D // 2), np.ones(D // 2)])
    A = (cos * w[None, :]).astype(np.float32)
    Bt = (sin * w_rot[None, :] * sgn[None, :]).astype(np.float32)
    if np.any(b != 0):
        raise NotImplementedError("nonzero qk-norm bias not supported")
    return A, Bt


def _center_heads(W):
    """Subtract per-head mean over output rows: W[o, c] - mean_{o' in head}."""
    W = W.reshape(H, D, C)
    return (W - W.mean(axis=1, keepdims=True)).reshape(C, C)


def kernel(**inputs):
    x = np.asarray(inputs["q"], dtype=np.float32)
    Wq = np.asarray(inputs["Wq"], dtype=np.float32)
    Wk = np.asarray(inputs["Wk"], dtype=np.float32)
    Wv = np.asarray(inputs["Wv"], dtype=np.float32)
    Wo = np.asarray(inputs["Wo"], dtype=np.float32)
    bo = np.asarray(inputs["bo"], dtype=np.float32)
    assert not np.any(bo != 0), "nonzero output bias not supported"

    Ak, Bk = _rope_tables(np.asarray(inputs["kn_w"], np.float32),
                          np.asarray(inputs["kn_b"], np.float32), L)
    Aq_full, Bq_full = _rope_tables(np.asarray(inputs["qn_w"], np.float32),
                                    np.asarray(inputs["qn_b"], np.float32), L)

    wq_bf = np.ascontiguousarray(_center_heads(Wq).T).astype(BF)   # [c, o]
    wk_bf = np.ascontiguousarray(_center_heads(Wk).T).astype(BF)
    wv_bf = np.ascontiguousarray(Wv.T).astype(BF)
    wo_bf = np.ascontiguousarray(Wo.T.reshape(NHP, P, C)).astype(BF)
    ones_pat_host = np.zeros((P, P), dtype=np.float32)
    ones_pat_host[64, 0:64] = 1.0
    ones_pat_host[32, 64:128] = 1.0

    if "nc" not in _NC_CACHE:
        _NC_CACHE["nc"] = _build_nc()
    nc = _NC_CACHE["nc"]

    # Each core sees its query half at x columns 0:512: roll the token axis
    # by -q0 (attention is order-invariant over keys; the k-side rope tables
    # are rolled identically so keys keep their true positions).
    in_maps = []
    for c in range(8):
        b_, half = c // 2, c % 2
        q0 = half * QL
        xTb = np.ascontiguousarray(np.roll(x[b_].T, -q0, axis=1)).astype(BF)
        in_maps.append({
            "xT": xTb,
            "wq": wq_bf, "wk": wk_bf, "wv": wv_bf, "wo": wo_bf,
            "aq": np.ascontiguousarray(Aq_full[q0:q0 + QL]),
            "bq": np.ascontiguousarray(Bq_full[q0:q0 + QL]),
            "ak": np.ascontiguousarray(np.roll(Ak, -q0, axis=0)),
            "bk": np.ascontiguousarray(np.roll(Bk, -q0, axis=0)),
            "onespat": ones_pat_host,
        })

    res = run_bass_kernel_spmd(nc, in_maps, core_ids=list(range(8)))
    out = np.empty((B, L, C), dtype=np.float32)
    for c in range(8):
        b_, half = c // 2, c % 2
        out[b_, half * QL:(half + 1) * QL, :] = res.results[c]["out"]
    return out


# revision 18
# speedup vs baseline: 1.9439x; 1.0869x over previous
"""MultiHeadAttention (qk-LayerNorm + RoPE) Trainium2 kernel, 8 NeuronCores.

Sharding: batch (4) x query-token-half (2x512 rows).  Core c handles batch
c//2 and output rows [512*(c%2), 512*(c%2)+512).  Each core computes K/V
projections for ALL 1024 tokens of its batch (duplicated within the pair --
cheaper than any collective: the baseline's ReduceScatter + device barrier
cost ~170us) and the Q projection for its 512 rows, so every core emits 512
complete output rows with ZERO cross-device communication.

Tricks:
- LN mean-centering is folded into Wq/Wk on the host (subtract per-head
  column means), so projections come out exactly mean-centered and only the
  variance is computed on-chip (ACT square + DVE reduce).
- rope(LN(x)) = inv * (x*A + rot(x)*B) since inv/shift are constant along d
  and shift == 0 -- one fused normalize+rope pass on DVE.
- D^-0.5 is folded into the q-side inv via bias = 64*eps, scale = 1 in the
  sqrt (giving 8*std directly).
- Attention per head-pair: scores for both heads land in one [128,1024]
  psum tile -> single 1024-wide exp.  V carries an appended ones column so
  ctx_unnorm and sum-of-exp come from one accumulated matmul; the odd head
  uses a 128-wide zero-padded V block so its ctx lands on psum partitions
  64:128 (sumexp on row 63), which keeps the o_proj contraction head-paired
  at K=128.
- All matmuls bf16 (PE upconverts to FP22 internally); psum f32.
"""
import sys

for _p in ("/opt/trn_rl_repo", "/root/.axon_site", "/root/.axon_site/_ro/trn_rl_repo",
           "/root/.axon_site/_ro/pypackages"):
    if _p not in sys.path:
        sys.path.append(_p)

import numpy as np
import ml_dtypes

import concourse.bass as bass
import concourse.tile as tile
from concourse import bacc, mybir
from concourse.bass_utils import run_bass_kernel_spmd
from concourse.masks import make_identity

F32 = mybir.dt.float32
F32R = mybir.dt.float32r
BF16 = mybir.dt.bfloat16
BF = ml_dtypes.bfloat16
P = 128
B, L, C, H, D = 4, 1024, 1024, 16, 64
NT = L // P        # 8 k/v token tiles
NQ = 4             # q token tiles (512 rows per core)
NCK = C // P       # 8 contraction chunks
NHP = H // 2       # 8 head pairs
QL = 512           # query rows per core
VW = 193           # v slot width per head pair: 65 (even) + 128 (odd)
THETA = 50000.0
EPS = 1e-5

_NC_CACHE = {}


def _build_nc():
    nc = bacc.Bacc("TRN2", target_bir_lowering=False, debug=False, num_devices=8)

    xT_d = nc.dram_tensor("xT", [C, L], BF16, kind="ExternalInput")
    wq_d = nc.dram_tensor("wq", [C, C], BF16, kind="ExternalInput")
    wk_d = nc.dram_tensor("wk", [C, C], BF16, kind="ExternalInput")
    wv_d = nc.dram_tensor("wv", [C, C], BF16, kind="ExternalInput")
    wo_d = nc.dram_tensor("wo", [NHP, P, C], BF16, kind="ExternalInput")
    aq_d = nc.dram_tensor("aq", [QL, D], F32, kind="ExternalInput")
    bq_d = nc.dram_tensor("bq", [QL, D], F32, kind="ExternalInput")
    ak_d = nc.dram_tensor("ak", [L, D], F32, kind="ExternalInput")
    bk_d = nc.dram_tensor("bk", [L, D], F32, kind="ExternalInput")
    onespat_d = nc.dram_tensor("onespat", [P, P], F32, kind="ExternalInput")
    out_d = nc.dram_tensor("out", [QL, C], F32, kind="ExternalOutput")

    with tile.TileContext(nc) as tc:
        with (
            tc.tile_pool(name="const", bufs=1) as constp,
            tc.tile_pool(name="w", bufs=1) as wpool,
            tc.tile_pool(name="big", bufs=1) as bigp,
            tc.tile_pool(name="sq", bufs=2) as sqp,
            tc.tile_pool(name="nrm", bufs=2) as nrmp,
            tc.tile_pool(name="st", bufs=2) as stp,
            tc.tile_pool(name="exp", bufs=3) as expp,
            tc.tile_pool(name="rb", bufs=2) as rbp,
            tc.tile_pool(name="rc", bufs=2) as rcp,
            tc.tile_pool(name="fin", bufs=2) as finp,
        ):
            ident = constp.tile([P, P], BF16)
            make_identity(nc, ident)
            eps_k = constp.tile([P, 1], F32)
            nc.vector.memset(eps_k[:], EPS)
            eps_q = constp.tile([P, 1], F32)
            nc.vector.memset(eps_q[:], float(D) * EPS)
            # ones_pat row 64 cols 0:64 = 1 (even-head recip broadcast),
            # row 32 cols 64:128 = 1 (odd-head recip broadcast to rows 64:128;
            # base partition must be 0/32/64 for the PE)
            ones_pat = constp.tile([P, P], F32R)
            nc.sync.dma_start(ones_pat[:], onespat_d.ap().bitcast(F32R))

            ak_t = constp.tile([P, NT, D], F32)
            nc.sync.dma_start(ak_t[:], ak_d.ap().rearrange("(t p) d -> p t d", p=P))
            bk_t = constp.tile([P, NT, D], F32)
            nc.sync.dma_start(bk_t[:], bk_d.ap().rearrange("(t p) d -> p t d", p=P))
            aq_t = constp.tile([P, NQ, D], F32)
            nc.sync.dma_start(aq_t[:], aq_d.ap().rearrange("(t p) d -> p t d", p=P))
            bq_t = constp.tile([P, NQ, D], F32)
            nc.sync.dma_start(bq_t[:], bq_d.ap().rearrange("(t p) d -> p t d", p=P))

            # per-ck chunks so matmuls start as soon as the first chunks land;
            # x arrives in 256-token column blocks so k-tile 0 is gated by
            # only wk + first x block (~2.5MB) instead of the full 6MB.
            def wchunks(d_, nm):
                lst = []
                for ck in range(NCK):
                    t_ = wpool.tile([P, C], BF16, tag=f"{nm}{ck}", name=f"{nm}{ck}")
                    nc.sync.dma_start(
                        t_[:], d_.ap().rearrange("(k p) o -> p k o", p=P)[:, ck, :])
                    lst.append(t_)
                return lst

            xr = xT_d.ap().rearrange("(k p) t -> p k t", p=P)
            xt_l = [[wpool.tile([P, 256], BF16, tag=f"xt{ck}_{tb}",
                                name=f"xt{ck}_{tb}") for tb in range(4)]
                    for ck in range(NCK)]

            def xchunks(tb):
                for ck in range(NCK):
                    nc.sync.dma_start(xt_l[ck][tb][:],
                                      xr[:, ck, tb * 256:(tb + 1) * 256])

            wk_l = wchunks(wk_d, "wk")
            xchunks(0)
            wv_l = wchunks(wv_d, "wv")
            xchunks(1)
            wq_l = wchunks(wq_d, "wq")
            xchunks(2)
            xchunks(3)
            wo_t = wpool.tile([P, NHP, C], BF16, name="wo")
            nc.sync.dma_start(wo_t[:], wo_d.ap().rearrange("g p o -> p g o"))

            def xsl(ck, tok):
                return xt_l[ck][tok // 256][:, tok % 256:tok % 256 + P]

            # v with per-pair slots [65 even | 128 odd]; odd block: col 32
            # ones, cols 64:128 = v, rest zero -> ctx lands on psum rows
            # 64:128 with sumexp on row 32.
            v_sb = bigp.tile([P, NT, NHP, VW], BF16)
            nc.vector.memset(v_sb[:, :, :, 65:129], 0.0)
            nc.vector.memset(v_sb[:, :, :, 64:65], 1.0)
            nc.vector.memset(v_sb[:, :, :, 97:98], 1.0)

            kT = bigp.tile([P, NHP, L], BF16)
            qT = bigp.tile([P, NHP, QL], BF16)
            ctxT = bigp.tile([P, NHP, QL], BF16)

            # ---------------- Phase 1: QKV + LN + RoPE + transpose ----------
            units = [("k", i) for i in range(NT)] + [("q", i) for i in range(NQ)]
            with tc.tile_pool(name="ps1", bufs=2, space="PSUM") as ps1, \
                 tc.tile_pool(name="pst", bufs=2, space="PSUM") as pst:
                for kind, ti in units:
                    is_k = kind == "k"
                    tok = ti * P
                    w_l = wk_l if is_k else wq_l
                    a_t, b_t = (ak_t, bk_t) if is_k else (aq_t, bq_t)
                    ps = ps1.tile([P, C], F32, tag="ps")
                    for m in range(2):
                        for ck in range(NCK):
                            nc.tensor.matmul(ps[:, bass.ts(m, 512)],
                                             xsl(ck, tok),
                                             w_l[ck][:, bass.ts(m, 512)],
                                             start=(ck == 0), stop=(ck == NCK - 1))
                    if is_k:
                        psv = ps1.tile([P, C], F32, tag="psv", bufs=1)
                        for m in range(2):
                            for ck in range(NCK):
                                nc.tensor.matmul(psv[:, bass.ts(m, 512)],
                                                 xsl(ck, tok),
                                                 wv_l[ck][:, bass.ts(m, 512)],
                                                 start=(ck == 0), stop=(ck == NCK - 1))
                    psr = ps[:].rearrange("p (h d) -> p h d", d=D)

                    # variance (mean is exactly 0: weights are pre-centered)
                    sq = sqp.tile([P, C], BF16, tag="sq")
                    nc.scalar.square(sq[:], ps[:])
                    vs = stp.tile([P, H], BF16, tag="vs")
                    with nc.allow_low_precision(reason="bf16 var accum, 0.4% rel"):
                        nc.vector.reduce_sum(
                            vs[:], sq[:].rearrange("p (h d) -> p h d", d=D),
                            axis=mybir.AxisListType.X)
                    std = stp.tile([P, H], F32, tag="std")
                    # k: sqrt(sum/64 + eps) = std ; q: sqrt(sum + 64eps) = 8*std
                    nc.scalar.activation(std[:], vs[:],
                                         mybir.ActivationFunctionType.Sqrt,
                                         bias=(eps_k[:] if is_k else eps_q[:]),
                                         scale=(1.0 / D if is_k else 1.0))
                    inv = stp.tile([P, H], F32, tag="inv")
                    nc.vector.reciprocal(inv[:], std[:])

                    # rope on raw centered ps, then scale by inv
                    nrm = nrmp.tile([P, H, D], BF16, tag="nrm")
                    tmp = nrmp.tile([P, H, D], BF16, tag="tmp", bufs=1)
                    a_b = a_t[:, ti, :].rearrange("p d -> p () d").to_broadcast((P, H, D))
                    nc.vector.tensor_mul(nrm[:], psr, a_b)
                    h_ = D // 2
                    nc.vector.tensor_mul(
                        tmp[:, :, 0:h_], psr[:, :, h_:D],
                        b_t[:, ti, 0:h_].rearrange("p d -> p () d").to_broadcast((P, H, h_)))
                    nc.vector.tensor_mul(
                        tmp[:, :, h_:D], psr[:, :, 0:h_],
                        b_t[:, ti, h_:D].rearrange("p d -> p () d").to_broadcast((P, H, h_)))
                    nc.vector.tensor_add(nrm[:], nrm[:], tmp[:])
                    inv_b = inv[:].rearrange("p h -> p h ()").to_broadcast((P, H, D))
                    nc.vector.tensor_mul(nrm[:], nrm[:], inv_b)

                    # transpose to [pair-channel, token]
                    dst = kT if is_k else qT
                    for g in range(2):
                        pt = pst.tile([P, 512], BF16, tag="pt")
                        for q4 in range(4):
                            hp = 4 * g + q4
                            nc.tensor.transpose(
                                pt[:, q4 * P:(q4 + 1) * P],
                                nrm[:, 2 * hp:2 * hp + 2, :].rearrange("p h d -> p (h d)"),
                                ident[:])
                        nc.scalar.copy(
                            dst[:, 4 * g:4 * g + 4, tok:tok + P],
                            pt[:].rearrange("p (f t) -> p f t", t=P))
                    if is_k:
                        psvr = psv[:].rearrange("p (g two d) -> p g two d", two=2, d=D)
                        nc.scalar.copy(v_sb[:, ti, :, 0:D], psvr[:, :, 0, :])
                        nc.scalar.copy(v_sb[:, ti, :, 129:VW], psvr[:, :, 1, :])

            # ---------------- Phase 2: attention per head pair --------------
            # The epilogue (recip -> rb broadcast -> normalize) is pipelined
            # one head-pair behind: the rb matmuls for hp are emitted after
            # hp+1's score/ctx matmuls, so the ~3.4us single-partition DVE
            # reciprocals never stall the in-order PE queue.
            with tc.tile_pool(name="pss", bufs=2, space="PSUM") as pssp, \
                 tc.tile_pool(name="psc", bufs=2, space="PSUM") as pscp:

                def epilogue_pe(hp, psc, rc):
                    rbps = pssp.tile([P, 2 * QL], F32, tag="pss")
                    nc.tensor.matmul(rbps[0:64, 0:QL], ones_pat[64:65, 0:64],
                                     rc[64:65, 0:QL], start=True, stop=True)
                    nc.tensor.matmul(rbps[:, QL:2 * QL], ones_pat[32:33, 0:P],
                                     rc[32:33, QL:2 * QL], start=True, stop=True)
                    rb = rbp.tile([P, 2 * QL], F32, tag="rb")
                    nc.vector.tensor_copy(rb[0:64, 0:QL], rbps[0:64, 0:QL])
                    nc.vector.tensor_copy(rb[64:128, QL:2 * QL],
                                          rbps[64:128, QL:2 * QL])
                    nc.vector.tensor_mul(ctxT[0:64, hp, :], psc[0:64, 0:QL],
                                         rb[0:64, 0:QL])
                    nc.vector.tensor_mul(ctxT[64:128, hp, :],
                                         psc[64:128, QL:2 * QL],
                                         rb[64:128, QL:2 * QL])

                pending = None
                for hp in range(NHP):
                    psc = pscp.tile([P, 2 * QL], F32, tag="psc")
                    for j in range(NT):
                        pss = pssp.tile([P, 2 * QL], F32, tag="pss")
                        jt = j * P
                        nc.tensor.matmul(pss[:, 0:QL],
                                         kT[0:64, hp, jt:jt + P], qT[0:64, hp, :],
                                         start=True, stop=True)
                        nc.tensor.matmul(pss[:, QL:2 * QL],
                                         kT[64:128, hp, jt:jt + P], qT[64:128, hp, :],
                                         start=True, stop=True)
                        ex = expp.tile([P, 2 * QL], BF16, tag="ex")
                        nc.scalar.activation(ex[:], pss[:],
                                             mybir.ActivationFunctionType.Exp)
                        nc.tensor.matmul(psc[0:65, 0:QL], v_sb[:, j, hp, 0:65],
                                         ex[:, 0:QL],
                                         start=(j == 0), stop=(j == NT - 1))
                        nc.tensor.matmul(psc[:, QL:2 * QL], v_sb[:, j, hp, 65:VW],
                                         ex[:, QL:2 * QL],
                                         start=(j == 0), stop=(j == NT - 1))
                    rc = rcp.tile([P, 2 * QL], F32R, tag="rc")
                    with nc.allow_low_precision(reason="f32r recip for rb matmul"):
                        nc.vector.reciprocal(rc[64:65, 0:QL], psc[64:65, 0:QL])
                        nc.vector.reciprocal(rc[32:33, QL:2 * QL],
                                             psc[32:33, QL:2 * QL])
                    if pending is not None:
                        epilogue_pe(*pending)
                    pending = (hp, psc, rc)
                epilogue_pe(*pending)

            # ---------------- Phase 3: output projection --------------------
            with tc.tile_pool(name="pso", bufs=2, space="PSUM") as psop:
                for qt in range(NQ):
                    pso = psop.tile([P, C], F32, tag="pso")
                    for m in range(2):
                        for g in range(NHP):
                            nc.tensor.matmul(pso[:, bass.ts(m, 512)],
                                             ctxT[:, g, qt * P:(qt + 1) * P],
                                             wo_t[:, g, bass.ts(m, 512)],
                                             start=(g == 0), stop=(g == NHP - 1))
                    osb = finp.tile([P, C], F32, tag="osb")
                    nc.vector.tensor_copy(osb[:], pso[:])
                    nc.sync.dma_start(out_d.ap()[qt * P:(qt + 1) * P, :], osb[:])

    nc.compile()
    return nc


def _rope_tables(w, b, n_tok):
    """A[t,d], B[t,d] with the rotate-half sign folded into B."""
    inv_freq = 1.0 / THETA ** (np.arange(0, D, 2, dtype=np.float64) / D)
    freqs = np.arange(n_tok, dtype=np.float64)[:, None] * inv_freq[None, :]
    freqs = np.concatenate([freqs, freqs], axis=1)           # [n_tok, D]
    cos, sin = np.cos(freqs), np.sin(freqs)
    w = w.astype(np.float64)
    w_rot = np.concatenate([w[D // 2:], w[:D // 2]])
    sgn = np.concatenate([-np.ones(D // 2), np.ones(D // 2)])
    A = (cos * w[None, :]).astype(np.float32)
    Bt = (sin * w_rot[None, :] * sgn[None, :]).astype(np.float32)
    if np.any(b != 0):
        raise NotImplementedError("nonzero qk-norm bias not supported")
    return A, Bt


def _center_heads(W):
    """Subtract per-head mean over output rows: W[o, c] - mean_{o' in head}."""
    W = W.reshape(H, D, C)
    return (W - W.mean(axis=1, keepdims=True)).reshape(C, C)


def kernel(**inputs):
    x = np.asarray(inputs["q"], dtype=np.float32)
    Wq = np.asarray(inputs["Wq"], dtype=np.float32)
    Wk = np.asarray(inputs["Wk"], dtype=np.float32)
    Wv = np.asarray(inputs["Wv"], dtype=np.float32)
    Wo = np.asarray(inputs["Wo"], dtype=np.float32)
    bo = np.asarray(inputs["bo"], dtype=np.float32)
    assert not np.any(bo != 0), "nonzero output bias not supported"

    Ak, Bk = _rope_tables(np.asarray(inputs["kn_w"], np.float32),
                          np.asarray(inputs["kn_b"], np.float32), L)
    Aq_full, Bq_full = _rope_tables(np.asarray(inputs["qn_w"], np.float32),
                                    np.asarray(inputs["qn_b"], np.float32), L)

    wq_bf = np.ascontiguousarray(_center_heads(Wq).T).astype(BF)   # [c, o]
    wk_bf = np.ascontiguousarray(_center_heads(Wk).T).astype(BF)
    wv_bf = np.ascontiguousarray(Wv.T).astype(BF)
    wo_bf = np.ascontiguousarray(Wo.T.reshape(NHP, P, C)).astype(BF)
    ones_pat_host = np.zeros((P, P), dtype=np.float32)
    ones_pat_host[64, 0:64] = 1.0
    ones_pat_host[32, 64:128] = 1.0

    if "nc" not in _NC_CACHE:
        _NC_CACHE["nc"] = _build_nc()
    nc = _NC_CACHE["nc"]

    # Each core sees its query half at x columns 0:512: roll the token axis
    # by -q0 (attention is order-invariant over keys; the k-side rope tables
    # are rolled identically so keys keep their true positions).
    in_maps = []
    for c in range(8):
        b_, half = c // 2, c % 2
        q0 = half * QL
        xTb = np.ascontiguousarray(np.roll(x[b_].T, -q0, axis=1)).astype(BF)
        in_maps.append({
            "xT": xTb,
            "wq": wq_bf, "wk": wk_bf, "wv": wv_bf, "wo": wo_bf,
            "aq": np.ascontiguousarray(Aq_full[q0:q0 + QL]),
            "bq": np.ascontiguousarray(Bq_full[q0:q0 + QL]),
            "ak": np.ascontiguousarray(np.roll(Ak, -q0, axis=0)),
            "bk": np.ascontiguousarray(np.roll(Bk, -q0, axis=0)),
            "onespat": ones_pat_host,
        })

    res = run_bass_kernel_spmd(nc, in_maps, core_ids=list(range(8)))
    out = np.empty((B, L, C), dtype=np.float32)
    for c in range(8):
        b_, half = c // 2, c % 2
        out[b_, half * QL:(half + 1) * QL, :] = res.results[c]["out"]
    return out
